# revision 27
# baseline (speedup 1.0000x reference)
"""Multi-head causal attention with RoPE on 8 Trainium2 NeuronCores.

Sharding: data-parallel over batch (B=2) x tensor-parallel over heads
(16 heads -> 4 groups of 4). Core c handles batch c//4, heads
[(c%4)*4, (c%4)*4+4). Each core computes a partial y = attn_out @ W_o
for its head group; the host sums the 4 partials per batch (the "W_o
all-reduce").

v4 schedule (vs the 3-serial-phase baseline): attention is ACT-bound
(~94us of exp on ScalarE) while its own PE work is only ~70us, so the
mb=1 projections + RoPE + V-transposes (~27us of PE) are interleaved
into the attention stream through 2 spare PSUM banks. Attention runs
SOLO-HEAD windows (sc 2x[128,1024] = 4 banks + acc [65,1024] = 2
banks) instead of head pairs, which is what frees those 2 banks.

  - Phase A-head: x^T + chunked weight DMAs (per-chunk contiguous
    DMAs so the first matmul starts ~2us in), proj K0/Q0/V0 (mb=0),
    RoPE(mb0), V0 transposes. ~30us, PE-dense.
  - Fused-B: 8 solo windows (head h, q-half pss); per ti: scores^T
    (K=128 zero-padded Q; K=64 streams measured 2.5x SLOWER, padding
    is free since matmul cost is N-column-bound), one exp on ACT,
    causal mask on GpSimd, PV one iteration behind (software
    pipeline). An aux generator paced ~2 steps/ti emits K1/Q1/V1
    projections into [128,512] PSUM chunks, piecewise RoPE(mb1)
    (GpSimd+DVE), and V1 transposes.
  - Phase C: y = onrm^T @ W_o per s-block; onrm is split into 3
    column-range tiles so early blocks only gate on their own range's
    norm chains; y streams out as bf16 (host upcasts and reduces).
"""

import os
import sys
from contextlib import ExitStack

import numpy as np

for _p in ("/opt/trn_rl_repo",):
    if os.path.isdir(_p) and _p not in sys.path:
        sys.path.insert(0, _p)

import ml_dtypes  # noqa: E402

BF16 = ml_dtypes.bfloat16

B, S, E = 2, 2048, 1024
H, DH = 16, 64
NCORES = 8
HPC = H // 4          # 4 heads per core
DC = HPC * DH         # 256 head dims per core
ATTN_SCALE = 1.0 / 32.0  # 1/sqrt(E)
ROPE_BASE = 10000.0
P = 128
NSB = S // P          # 16 sequence blocks
NEC = E // P          # 8 E chunks
MB = DC // P          # 2 partition blocks of head dims

_PROG = None


def _perm64():
    """perm[j] = original head-dim index stored at permuted position j.

    Quadrant q of the permuted layout holds RoPE pairs i in
    [16q, 16q+16): even elements (2i) at slots 0-15, odd (2i+1) at
    slots 16-31. The rotation partner is then always +-16 partitions
    away within one 32-partition quadrant (stream_shuffle range).
    """
    j = np.arange(64)
    qd, r = j // 32, j % 32
    i = 16 * qd + (r % 16)
    return 2 * i + (r >= 16)


def _cos_sin_tiles():
    pl = np.arange(P) % 64
    qd, r = pl // 32, pl % 32
    i = 16 * qd + (r % 16)
    inv = ROPE_BASE ** (-(2.0 * i) / DH)
    ang = np.arange(S)[None, :] * inv[:, None]          # (128, S)
    sgn = np.where(r < 16, -1.0, 1.0)[:, None]
    return ang, sgn


def _build_program():
    import concourse.bacc as bacc
    import concourse.tile as tile
    from concourse import masks, mybir

    f32 = mybir.dt.float32
    bf16 = mybir.dt.bfloat16
    AF = mybir.ActivationFunctionType

    nc = bacc.Bacc("TRN2", target_bir_lowering=False, debug=False)
    xbt = nc.dram_tensor("xbt", [E, S], bf16, kind="ExternalInput").ap()
    wq = nc.dram_tensor("wq", [E, DC], bf16, kind="ExternalInput").ap()
    wk = nc.dram_tensor("wk", [E, DC], bf16, kind="ExternalInput").ap()
    wv = nc.dram_tensor("wv", [E, DC], bf16, kind="ExternalInput").ap()
    wo = nc.dram_tensor("wo", [DC, E], bf16, kind="ExternalInput").ap()
    cosr = nc.dram_tensor("cosr", [P, S], bf16, kind="ExternalInput").ap()
    sinr = nc.dram_tensor("sinr", [P, S], bf16, kind="ExternalInput").ap()
    cmask = nc.dram_tensor("cmask", [P, P], bf16, kind="ExternalInput").ap()
    y = nc.dram_tensor("y", [S, E], bf16, kind="ExternalOutput").ap()

    with ExitStack() as ctx:
        tc = ctx.enter_context(tile.TileContext(nc))
        consts = ctx.enter_context(tc.tile_pool(name="consts", bufs=1))
        persist = ctx.enter_context(tc.tile_pool(name="persist", bufs=1))

        # per-mb tiles (split so fused-B's mb=1 writes never create
        # false tile-granular deps against mb=0 reads)
        qcT = [persist.tile([P, S], bf16, tag=f"qcT{m}", name=f"qcT{m}")
               for m in range(MB)]
        kcT = [persist.tile([P, S], bf16, tag=f"kcT{m}", name=f"kcT{m}")
               for m in range(MB)]
        vT = [persist.tile([P, S], bf16, tag=f"vT{m}", name=f"vT{m}")
              for m in range(MB)]
        kT = [persist.tile([P, S], bf16, tag=f"kT{m}", name=f"kT{m}")
              for m in range(MB)]
        # qz: RoPE'd Q^T zero-padded per head parity (par p: head
        # 2mb+p's 64 rows live, other 64 rows zero) so scores use the
        # full K=128 contraction.
        qz = [persist.tile([P, 2, S], bf16, tag=f"qz{m}", name=f"qz{m}")
              for m in range(MB)]
        # V natural layout + ones column at col 64; one tile per
        # (mb, sb, par) so PV at ti only gates on its own block's
        # transpose, and so the xbar transpose DMA gets a contiguous
        # offset-0 [128, 64] destination (strided dsts mis-write)
        vn = [[[persist.tile([P, 65], bf16, tag=f"vn{m}_{sb}_{p_}",
                             name=f"vn{m}_{sb}_{p_}")
                for p_ in range(2)]
               for sb in range(NSB)] for m in range(MB)]
        # onrm split by epilogue chunk boundaries so phase C's
        # per-s-block matmuls gate only on their own q-range's chains
        onrm_parts = [
            (0, 1024,
             persist.tile([P, MB, 1024], bf16, tag="onrm0", name="onrm0")),
            (1024, 1536,
             persist.tile([P, MB, 512], bf16, tag="onrm1", name="onrm1")),
            (1536, 2048,
             persist.tile([P, MB, 512], bf16, tag="onrm2", name="onrm2")),
        ]

        def onrm_ap(pl, ph, mb, c0, c1):
            for lo_, hi_, t_ in onrm_parts:
                if c0 >= lo_ and c1 <= hi_:
                    return t_[pl:ph, mb, c0 - lo_:c1 - lo_]
            raise AssertionError((c0, c1))

        xp = ctx.enter_context(tc.tile_pool(name="xp", bufs=1))
        xT = xp.tile([P, NEC, S], bf16, tag="xT")

        wk_t = consts.tile([P, NEC, DC], bf16, tag="wk")
        wq_t = consts.tile([P, NEC, DC], bf16, tag="wq")
        wv_t = consts.tile([P, NEC, DC], bf16, tag="wv")
        cos_t = consts.tile([P, S], bf16, tag="cos")
        sin_t = consts.tile([P, S], bf16, tag="sin")
        msk_t = consts.tile([P, P], bf16, tag="msk")
        wo_t = consts.tile([P, MB, E], bf16, tag="wo")

        shuf_mask = list(range(16, 32)) + list(range(16))

        # ---- Phase A-head: DMAs, mb=0 projections, RoPE0, vn0 ----
        with ExitStack() as actx:
            pr_ps = actx.enter_context(
                tc.tile_pool(name="pr_ps", bufs=2, space="PSUM")
            )
            rtmp = actx.enter_context(tc.tile_pool(name="rtmp", bufs=2))

            # chunked DMAs: wk[c] || x[c] across both rings so the
            # first proj matmul (wk chunk 0 + x chunk 0) starts ~2us in
            nc.scalar.dma_start(wk_t[:, 0, :], wk[0:P, :])
            for ec in range(NEC):
                eng = nc.sync if ec % 2 == 0 else nc.scalar
                eng.dma_start(xT[:, ec, :], xbt[ec * P:(ec + 1) * P, :])
                if ec > 0:
                    eng2 = nc.scalar if ec % 2 == 0 else nc.sync
                    eng2.dma_start(wk_t[:, ec, :], wk[ec * P:(ec + 1) * P, :])
            nc.sync.dma_start(cos_t[:], cosr)
            nc.scalar.dma_start(sin_t[:], sinr)
            for ec in range(NEC):
                eng = nc.sync if ec % 2 == 0 else nc.scalar
                eng.dma_start(wq_t[:, ec, :], wq[ec * P:(ec + 1) * P, :])
            nc.scalar.dma_start(msk_t[:], cmask)
            for ec in range(NEC):
                eng = nc.scalar if ec % 2 == 0 else nc.sync
                eng.dma_start(wv_t[:, ec, :], wv[ec * P:(ec + 1) * P, :])
            nc.sync.dma_start(wo_t[:], wo.rearrange("(c p) n -> p c n", p=P))

            for m in range(MB):
                nc.gpsimd.memset(qz[m][0:DH, 1, :], 0.0)
                nc.gpsimd.memset(qz[m][DH:P, 0, :], 0.0)
            for m in range(MB):
                for sb in range(NSB):
                    for p_ in range(2):
                        nc.vector.memset(vn[m][sb][p_][:, 64:65], 1.0)
            # preload the exp table set so the ~2.7us ACT_TABLE_LOAD
            # is not on fused-B's critical path
            warm = rtmp.tile([1, P], f32, tag="warm", name="warm")
            nc.scalar.activation(warm[:], cos_t[0:1, 0:P], AF.Exp)

            def proj0(wt, dst, nm):
                for half in range(2):
                    ps = pr_ps.tile([P, S // 2], f32, tag="proj",
                                    name=f"pj0{nm}_{half}")
                    for i in range(NEC):
                        for qt in range(2):
                            c0 = half * 1024 + qt * 512
                            nc.tensor.matmul(
                                ps[:, qt * 512:(qt + 1) * 512],
                                lhsT=wt[:, i, 0:P],
                                rhs=xT[:, i, c0:c0 + 512],
                                start=(i == 0),
                                stop=(i == NEC - 1),
                            )
                    nc.scalar.copy(
                        dst[:, half * 1024:(half + 1) * 1024], ps[:]
                    )

            def rope0(src, kdst, qdst):
                # mb=0 RoPE on DVE (idle during PE-bound phase A);
                # kdst: full-width K output; qdst: zero-padded Q planes
                sh = rtmp.tile([P, S], bf16, tag="shuf",
                               name=f"sh0{'k' if qdst is None else 'q'}")
                nc.vector.stream_shuffle(sh[:], src[:], shuf_mask)
                nc.vector.tensor_mul(sh[:], sh[:], sin_t[:])
                if qdst is None:
                    nc.vector.tensor_mul(kdst[:], src[:], cos_t[:])
                    nc.vector.tensor_add(kdst[:], kdst[:], sh[:])
                else:
                    for par in range(2):
                        o0 = par * DH
                        nc.vector.tensor_mul(
                            qdst[o0:o0 + DH, par, :],
                            src[o0:o0 + DH, :],
                            cos_t[o0:o0 + DH, :],
                        )
                        nc.vector.tensor_add(
                            qdst[o0:o0 + DH, par, :],
                            qdst[o0:o0 + DH, par, :],
                            sh[o0:o0 + DH, :],
                        )

            proj0(wk_t, kcT[0], "k")
            rope0(kcT[0], kT[0], None)
            proj0(wq_t, qcT[0], "q")
            rope0(qcT[0], None, qz[0])
            proj0(wv_t, vT[0], "v")
            # V0 natural layout via the DMA xbar transpose engine (the
            # rings are idle by now; costs no PE/PSUM/DVE)
            for sb in range(NSB):
                for par in range(2):
                    nc.sync.dma_start_transpose(
                        vn[0][sb][par][:, 0:64],
                        vT[0][par * DH:(par + 1) * DH, sb * P:(sb + 1) * P],
                    )

        # ---- Fused-B: solo-head attention windows + aux interleave ----
        with ExitStack() as bctx:
            sc_ps = bctx.enter_context(
                tc.tile_pool(name="sc_ps", bufs=2, space="PSUM")
            )
            ac_ps = bctx.enter_context(
                tc.tile_pool(name="ac_ps", bufs=1, space="PSUM")
            )
            aux_ps = bctx.enter_context(
                tc.tile_pool(name="aux_ps", bufs=2, space="PSUM")
            )
            ptp = bctx.enter_context(tc.tile_pool(name="ptp", bufs=4))
            dn = bctx.enter_context(tc.tile_pool(name="dn", bufs=2))
            rb = bctx.enter_context(tc.tile_pool(name="rb", bufs=2))

            def aux_steps():
                # mb=1 projections in [128,512] PSUM chunks (the 2
                # spare banks), RoPE(mb1) piecewise, V1 transposes.
                # Each yield is <= ~1us of engine time so attention
                # emission never stalls behind a big burst.
                for wt, dst, nm in ((wk_t, kcT[1], "k1"),
                                    (wq_t, qcT[1], "q1"),
                                    (wv_t, vT[1], "v1")):
                    for half in range(2):
                        for qt in range(2):
                            c0 = half * 1024 + qt * 512
                            ps = aux_ps.tile([P, 512], f32, tag="auxp",
                                             name=f"pj_{nm}_{c0}")
                            for i0 in (0, 4):
                                for i in range(i0, i0 + 4):
                                    nc.tensor.matmul(
                                        ps[:],
                                        lhsT=wt[:, i, P:2 * P],
                                        rhs=xT[:, i, c0:c0 + 512],
                                        start=(i == 0),
                                        stop=(i == NEC - 1),
                                    )
                                yield
                            nc.vector.tensor_copy(
                                dst[:, c0:c0 + 512], ps[:]
                            )
                            yield
                    if nm == "k1":
                        for c0 in range(0, S, 512):
                            sh = rb.tile([P, 512], bf16, tag="rsh",
                                         name=f"rshk{c0}")
                            nc.vector.stream_shuffle(
                                sh[:], kcT[1][:, c0:c0 + 512], shuf_mask
                            )
                            yield
                            nc.gpsimd.tensor_mul(
                                sh[:], sh[:], sin_t[:, c0:c0 + 512]
                            )
                            yield
                            nc.gpsimd.tensor_mul(
                                kT[1][:, c0:c0 + 512],
                                kcT[1][:, c0:c0 + 512],
                                cos_t[:, c0:c0 + 512],
                            )
                            yield
                            nc.gpsimd.tensor_add(
                                kT[1][:, c0:c0 + 512],
                                kT[1][:, c0:c0 + 512],
                                sh[:],
                            )
                            yield
                    elif nm == "q1":
                        for c0 in range(0, S, 512):
                            sh = rb.tile([P, 512], bf16, tag="rsh",
                                         name=f"rshq{c0}")
                            nc.vector.stream_shuffle(
                                sh[:], qcT[1][:, c0:c0 + 512], shuf_mask
                            )
                            yield
                            nc.vector.tensor_mul(
                                sh[:], sh[:], sin_t[:, c0:c0 + 512]
                            )
                            yield
                            for par in range(2):
                                o0 = par * DH
                                nc.vector.tensor_mul(
                                    qz[1][o0:o0 + DH, par, c0:c0 + 512],
                                    qcT[1][o0:o0 + DH, c0:c0 + 512],
                                    cos_t[o0:o0 + DH, c0:c0 + 512],
                                )
                                nc.vector.tensor_add(
                                    qz[1][o0:o0 + DH, par, c0:c0 + 512],
                                    qz[1][o0:o0 + DH, par, c0:c0 + 512],
                                    sh[o0:o0 + DH, :],
                                )
                                yield
                for sb in range(NSB):
                    # V1 natural layout via DMA xbar transpose on the
                    # sync ring (idle during fused-B)
                    for par in range(2):
                        nc.sync.dma_start_transpose(
                            vn[1][sb][par][:, 0:64],
                            vT[1][par * DH:(par + 1) * DH,
                                  sb * P:(sb + 1) * P],
                        )
                        yield

            aux = aux_steps()
            aux_state = [True]

            def pump(n):
                for _ in range(n):
                    if not aux_state[0]:
                        return
                    try:
                        next(aux)
                    except StopIteration:
                        aux_state[0] = False

            def epilogue_copies(h, acc, c0, c1):
                # stage out^T + the denominator row out of PSUM (plain
                # DVE copies; these free the accumulator banks)
                q0 = (c0 // 1024) * 1024
                w = c1 - c0
                l0 = c0 - q0
                acb = dn.tile([DH, w], f32, tag="acb", name=f"acb{h}_{c0}")
                nc.vector.tensor_copy(acb[:], acc[0:DH, l0:l0 + w])
                den0 = dn.tile([1, w], f32, tag="den0", name=f"den0{h}_{c0}")
                nc.vector.tensor_copy(den0[:], acc[64:65, l0:l0 + w])
                return h, c0, c1, acb, den0

            def epilogue_norm(h, c0, c1, acb, den0):
                # single-pass approx reciprocal (~18 bits), partition
                # broadcast on GpSimd (reads partition 0 => rden is a
                # base-0 tile), one multiply
                mb, off = h // 2, (h % 2) * DH
                w = c1 - c0
                rden = dn.tile([1, w], f32, tag="rden", name=f"rden{h}_{c0}")
                nc.vector.reciprocal_approx_fast(rden[:], den0[:])
                rdb = dn.tile([DH, w], f32, tag="rdb", name=f"rdb{h}_{c0}")
                nc.gpsimd.partition_broadcast(rdb[:], rden[:])
                nc.vector.tensor_mul(
                    onrm_ap(off, off + DH, mb, c0, c1), acb[:], rdb[:]
                )

            deferred_norms = []
            for widx in range(8):
                h, pss = widx // 2, widx % 2
                mb, par = h // 2, h % 2
                q0 = pss * 1024
                acc = ac_ps.tile([65, 1024], f32, tag="acc",
                                 name=f"acc_{h}_{pss}")

                def issue_pv(ti, pt, lo, hi, acc=acc, h=h, mb=mb, q0=q0):
                    p0 = lo
                    while p0 < hi:
                        bk = p0 // 512
                        p1 = min(hi, (bk + 1) * 512)
                        nc.tensor.matmul(
                            acc[:, p0 - q0:p1 - q0],
                            lhsT=vn[mb][ti][h % 2][:],
                            rhs=pt[:, p0 - q0:p1 - q0],
                            start=(ti == 0),
                            stop=(ti == 4 * bk + 3),
                        )
                        p0 = p1

                pending = None
                nti = 8 if pss == 0 else NSB
                for ti in range(nti):
                    if ti == 2 and deferred_norms:
                        # previous window's normalize chains, emitted
                        # here so the PSUM-release semaphores are not
                        # queued behind the reciprocal work
                        for st in deferred_norms:
                            epilogue_norm(*st)
                        deferred_norms = []
                    if pss == 1 and ti == 13:
                        # acc bank for cols q0..q0+512 got its last PV
                        # at ti==11: normalize it mid-loop
                        epilogue_norm(
                            *epilogue_copies(h, acc, q0, q0 + 512)
                        )
                    pump(2 if widx < 4 else 1)
                    t0 = ti * P
                    lo = max(t0, q0)
                    hi = q0 + 1024
                    sc = sc_ps.tile([P, 1024], f32, tag="sc",
                                    name=f"sc_{h}_{pss}_{ti}")
                    p0 = lo
                    while p0 < hi:
                        p1 = min(hi, (p0 // 512 + 1) * 512)
                        nc.tensor.matmul(
                            sc[:, p0 - q0:p1 - q0],
                            lhsT=kT[mb][:, t0:t0 + P],
                            rhs=qz[mb][:, par, p0:p1],
                        )
                        p0 = p1
                    pt = ptp.tile([P, 1024], bf16, tag="pt",
                                  name=f"pt_{h}_{pss}_{ti}")
                    nc.scalar.activation(
                        pt[:, lo - q0:hi - q0],
                        sc[:, lo - q0:hi - q0],
                        AF.Exp,
                        scale=ATTN_SCALE,
                    )
                    if t0 >= q0:
                        # diagonal-block causal mask on GpSimd (full
                        # 128 partitions, free-dim offset: gpsimd-safe)
                        nc.gpsimd.tensor_mul(
                            pt[:, t0 - q0:t0 - q0 + P],
                            pt[:, t0 - q0:t0 - q0 + P],
                            msk_t[:],
                        )
                    if pending is not None:
                        issue_pv(*pending)
                    pending = (ti, pt, lo, hi)
                issue_pv(*pending)
                if pss == 1:
                    chunks = [(q0 + 512, q0 + 1024)]
                else:
                    chunks = [(q0, q0 + 1024)]
                staged = [epilogue_copies(h, acc, c0, c1)
                          for c0, c1 in chunks]
                if widx == 7:
                    for st in staged:
                        epilogue_norm(*st)
                else:
                    deferred_norms = staged
            pump(1000)  # drain any leftover aux work

        # ---- Phase C: output projection ----
        with ExitStack() as cctx:
            y_ps = cctx.enter_context(
                tc.tile_pool(name="y_ps", bufs=2, space="PSUM")
            )
            yo = cctx.enter_context(tc.tile_pool(name="yo", bufs=3))
            for sb_i in range(NSB):
                yp = y_ps.tile([P, E], f32, tag="yp")
                for mb in range(MB):
                    for half in range(2):
                        nc.tensor.matmul(
                            yp[:, half * 512:(half + 1) * 512],
                            lhsT=onrm_ap(0, P, mb, sb_i * P, (sb_i + 1) * P),
                            rhs=wo_t[:, mb, half * 512:(half + 1) * 512],
                            start=(mb == 0),
                            stop=(mb == MB - 1),
                        )
                ys = yo.tile([P, E], bf16, tag="ys")
                for half in range(2):
                    sl = slice(half * 512, (half + 1) * 512)
                    if (sb_i + half) % 2 == 0:
                        nc.vector.tensor_copy(ys[:, sl], yp[:, sl])
                    else:
                        nc.scalar.copy(ys[:, sl], yp[:, sl])
                    eng = nc.sync if half == 0 else nc.scalar
                    eng.dma_start(y[sb_i * P:(sb_i + 1) * P, sl], ys[:, sl])

    nc.compile()
    return nc


def get_program():
    global _PROG
    if _PROG is None:
        _PROG = _build_program()
    return _PROG


def make_in_maps(x, W_q, W_k, W_v, W_o):
    perm = _perm64()
    idx_local = (np.arange(DC) // 64) * 64 + perm[np.arange(DC) % 64]
    ang, sgn = _cos_sin_tiles()
    cos_np = np.cos(ang).astype(BF16)
    sin_np = (sgn * np.sin(ang)).astype(BF16)
    # scores tile is (t, q): keep t <= q -> upper triangular incl. diagonal
    cmask_np = np.triu(np.ones((P, P))).astype(BF16)
    in_maps = []
    for c in range(NCORES):
        b, hg = c // 4, c % 4
        base = hg * DC
        in_maps.append(
            dict(
                xbt=np.ascontiguousarray(x[b].T.astype(BF16)),
                wq=np.ascontiguousarray(W_q[:, base + idx_local].astype(BF16)),
                wk=np.ascontiguousarray(W_k[:, base + idx_local].astype(BF16)),
                wv=np.ascontiguousarray(W_v[:, base:base + DC].astype(BF16)),
                wo=np.ascontiguousarray(W_o[base:base + DC, :].astype(BF16)),
                cosr=cos_np,
                sinr=sin_np,
                cmask=cmask_np,
            )
        )
    return in_maps


def kernel(x, W_q, W_k, W_v, W_o, _trace=False, _trace_cores=None):
    from concourse.bass_utils import run_bass_kernel_spmd

    x = np.asarray(x, dtype=np.float32)
    W_q = np.asarray(W_q, dtype=np.float32)
    W_k = np.asarray(W_k, dtype=np.float32)
    W_v = np.asarray(W_v, dtype=np.float32)
    W_o = np.asarray(W_o, dtype=np.float32)

    nc = get_program()
    in_maps = make_in_maps(x, W_q, W_k, W_v, W_o)
    res = run_bass_kernel_spmd(
        nc,
        in_maps,
        list(range(NCORES)),
        trace=_trace,
        trace_cores=_trace_cores,
    )
    y = np.zeros((B, S, E), np.float32)
    for c in range(NCORES):
        y[c // 4] += res.results[c]["y"].astype(np.float32)
    if _trace:
        return y, res
    return y


# revision 34
# speedup vs baseline: 1.2380x; 1.2380x over previous
"""Multi-head causal attention with RoPE on 8 Trainium2 NeuronCores.

Sharding: data-parallel over batch (B=2) x tensor-parallel over heads
(16 heads -> 4 groups of 4). Core c handles batch c//4, heads
[(c%4)*4, (c%4)*4+4). Each core computes a partial y = attn_out @ W_o
for its head group; the host sums the 4 partials per batch (the "W_o
all-reduce").

v4 schedule (vs the 3-serial-phase baseline): attention is ACT-bound
(~94us of exp on ScalarE) while its own PE work is only ~70us, so the
mb=1 projections + RoPE + V-transposes (~27us of PE) are interleaved
into the attention stream through 2 spare PSUM banks. Attention runs
SOLO-HEAD windows (sc 2x[128,1024] = 4 banks + acc [65,1024] = 2
banks) instead of head pairs, which is what frees those 2 banks.

  - Phase A-head: x^T + chunked weight DMAs (per-chunk contiguous
    DMAs so the first matmul starts ~2us in), proj K0/Q0/V0 (mb=0),
    RoPE(mb0), V0 transposes. ~30us, PE-dense.
  - Fused-B: 8 solo windows (head h, q-half pss); per ti: scores^T
    (K=128 zero-padded Q; K=64 streams measured 2.5x SLOWER, padding
    is free since matmul cost is N-column-bound), one exp on ACT,
    causal mask on GpSimd, PV one iteration behind (software
    pipeline). An aux generator paced ~2 steps/ti emits K1/Q1/V1
    projections into [128,512] PSUM chunks, piecewise RoPE(mb1)
    (GpSimd+DVE), and V1 transposes.
  - Phase C: y = onrm^T @ W_o per s-block; onrm is split into 3
    column-range tiles so early blocks only gate on their own range's
    norm chains; y streams out as bf16 (host upcasts and reduces).
"""

import os
import sys
from contextlib import ExitStack

import numpy as np

for _p in ("/opt/trn_rl_repo",):
    if os.path.isdir(_p) and _p not in sys.path:
        sys.path.insert(0, _p)

import ml_dtypes  # noqa: E402

BF16 = ml_dtypes.bfloat16

B, S, E = 2, 2048, 1024
H, DH = 16, 64
NCORES = 8
HPC = H // 4          # 4 heads per core
DC = HPC * DH         # 256 head dims per core
ATTN_SCALE = 1.0 / 32.0  # 1/sqrt(E)
ROPE_BASE = 10000.0
P = 128
NSB = S // P          # 16 sequence blocks
NEC = E // P          # 8 E chunks
MB = DC // P          # 2 partition blocks of head dims

_PROG = None


def _perm64():
    """perm[j] = original head-dim index stored at permuted position j.

    Quadrant q of the permuted layout holds RoPE pairs i in
    [16q, 16q+16): even elements (2i) at slots 0-15, odd (2i+1) at
    slots 16-31. The rotation partner is then always +-16 partitions
    away within one 32-partition quadrant (stream_shuffle range).
    """
    j = np.arange(64)
    qd, r = j // 32, j % 32
    i = 16 * qd + (r % 16)
    return 2 * i + (r >= 16)


def _cos_sin_tiles():
    pl = np.arange(P) % 64
    qd, r = pl // 32, pl % 32
    i = 16 * qd + (r % 16)
    inv = ROPE_BASE ** (-(2.0 * i) / DH)
    ang = np.arange(S)[None, :] * inv[:, None]          # (128, S)
    sgn = np.where(r < 16, -1.0, 1.0)[:, None]
    return ang, sgn


def _build_program():
    import concourse.bacc as bacc
    import concourse.tile as tile
    from concourse import masks, mybir

    f32 = mybir.dt.float32
    bf16 = mybir.dt.bfloat16
    AF = mybir.ActivationFunctionType

    nc = bacc.Bacc("TRN2", target_bir_lowering=False, debug=False)
    xbt = nc.dram_tensor("xbt", [E, S], bf16, kind="ExternalInput").ap()
    wq = nc.dram_tensor("wq", [E, DC], bf16, kind="ExternalInput").ap()
    wk = nc.dram_tensor("wk", [E, DC], bf16, kind="ExternalInput").ap()
    wv = nc.dram_tensor("wv", [E, DC], bf16, kind="ExternalInput").ap()
    wo = nc.dram_tensor("wo", [DC, E], bf16, kind="ExternalInput").ap()
    cosr = nc.dram_tensor("cosr", [P, S], bf16, kind="ExternalInput").ap()
    sinr = nc.dram_tensor("sinr", [P, S], bf16, kind="ExternalInput").ap()
    cmask = nc.dram_tensor("cmask", [P, P], bf16, kind="ExternalInput").ap()
    y = nc.dram_tensor("y", [S, E], bf16, kind="ExternalOutput").ap()

    with ExitStack() as ctx:
        tc = ctx.enter_context(tile.TileContext(nc))
        consts = ctx.enter_context(tc.tile_pool(name="consts", bufs=1))
        persist = ctx.enter_context(tc.tile_pool(name="persist", bufs=1))

        ident = consts.tile([P, P], bf16, tag="ident")
        masks.make_identity(nc, ident[:])

        # per-mb tiles (split so fused-B's mb=1 writes never create
        # false tile-granular deps against mb=0 reads)
        qcT = [persist.tile([P, S], bf16, tag=f"qcT{m}", name=f"qcT{m}")
               for m in range(MB)]
        kcT = [persist.tile([P, S], bf16, tag=f"kcT{m}", name=f"kcT{m}")
               for m in range(MB)]
        vT = [persist.tile([P, S], bf16, tag=f"vT{m}", name=f"vT{m}")
              for m in range(MB)]
        kT = [persist.tile([P, S], bf16, tag=f"kT{m}", name=f"kT{m}")
              for m in range(MB)]
        # qz: RoPE'd Q^T zero-padded per head parity (par p: head
        # 2mb+p's 64 rows live, other 64 rows zero) so scores use the
        # full K=128 contraction.
        qz = [persist.tile([P, 2, S], bf16, tag=f"qz{m}", name=f"qz{m}")
              for m in range(MB)]
        # V natural layout + ones column at col 64; one tile per
        # (mb, sb, par) so PV at ti only gates on its own block's
        # transpose, and so the xbar transpose DMA gets a contiguous
        # offset-0 [128, 64] destination (strided dsts mis-write)
        vn = [[[persist.tile([P, 65], bf16, tag=f"vn{m}_{sb}_{p_}",
                             name=f"vn{m}_{sb}_{p_}")
                for p_ in range(2)]
               for sb in range(NSB)] for m in range(MB)]
        # onrm split by epilogue chunk boundaries so phase C's
        # per-s-block matmuls gate only on their own q-range's chains
        onrm_parts = [
            (0, 1024,
             persist.tile([P, MB, 1024], bf16, tag="onrm0", name="onrm0")),
            (1024, 1536,
             persist.tile([P, MB, 512], bf16, tag="onrm1", name="onrm1")),
            (1536, 2048,
             persist.tile([P, MB, 512], bf16, tag="onrm2", name="onrm2")),
        ]

        def onrm_ap(pl, ph, mb, c0, c1):
            for lo_, hi_, t_ in onrm_parts:
                if c0 >= lo_ and c1 <= hi_:
                    return t_[pl:ph, mb, c0 - lo_:c1 - lo_]
            raise AssertionError((c0, c1))

        xp = ctx.enter_context(tc.tile_pool(name="xp", bufs=1))
        xT = xp.tile([P, NEC, S], bf16, tag="xT")

        wk_t = consts.tile([P, NEC, DC], bf16, tag="wk")
        wq_t = consts.tile([P, NEC, DC], bf16, tag="wq")
        wv_t = consts.tile([P, NEC, DC], bf16, tag="wv")
        cos_t = consts.tile([P, S], bf16, tag="cos")
        sin_t = consts.tile([P, S], bf16, tag="sin")
        msk_t = consts.tile([P, P], bf16, tag="msk")
        wo_t = consts.tile([P, MB, E], bf16, tag="wo")

        shuf_mask = list(range(16, 32)) + list(range(16))

        # ---- Phase A-head: DMAs, mb=0 projections, RoPE0, vn0 ----
        with ExitStack() as actx:
            pr_ps = actx.enter_context(
                tc.tile_pool(name="pr_ps", bufs=2, space="PSUM")
            )
            rtmp = actx.enter_context(tc.tile_pool(name="rtmp", bufs=2))

            # chunked DMAs: wk[c] || x[c] across both rings so the
            # first proj matmul (wk chunk 0 + x chunk 0) starts ~2us in
            nc.scalar.dma_start(wk_t[:, 0, :], wk[0:P, :])
            for ec in range(NEC):
                eng = nc.sync if ec % 2 == 0 else nc.scalar
                eng.dma_start(xT[:, ec, :], xbt[ec * P:(ec + 1) * P, :])
                if ec > 0:
                    eng2 = nc.scalar if ec % 2 == 0 else nc.sync
                    eng2.dma_start(wk_t[:, ec, :], wk[ec * P:(ec + 1) * P, :])
            nc.sync.dma_start(cos_t[:], cosr)
            nc.scalar.dma_start(sin_t[:], sinr)
            for ec in range(NEC):
                eng = nc.sync if ec % 2 == 0 else nc.scalar
                eng.dma_start(wq_t[:, ec, :], wq[ec * P:(ec + 1) * P, :])
            nc.scalar.dma_start(msk_t[:], cmask)
            for ec in range(NEC):
                eng = nc.scalar if ec % 2 == 0 else nc.sync
                eng.dma_start(wv_t[:, ec, :], wv[ec * P:(ec + 1) * P, :])
            nc.sync.dma_start(wo_t[:], wo.rearrange("(c p) n -> p c n", p=P))

            for m in range(MB):
                nc.gpsimd.memset(qz[m][0:DH, 1, :], 0.0)
                nc.gpsimd.memset(qz[m][DH:P, 0, :], 0.0)
            for m in range(MB):
                for sb in range(NSB):
                    for p_ in range(2):
                        nc.vector.memset(vn[m][sb][p_][:, 64:65], 1.0)
            # preload the exp table set so the ~2.7us ACT_TABLE_LOAD
            # is not on fused-B's critical path
            warm = rtmp.tile([1, P], f32, tag="warm", name="warm")
            nc.scalar.activation(warm[:], cos_t[0:1, 0:P], AF.Exp)

            def proj0(wt, dst, nm):
                for half in range(2):
                    ps = pr_ps.tile([P, S // 2], f32, tag="proj",
                                    name=f"pj0{nm}_{half}")
                    for i in range(NEC):
                        for qt in range(2):
                            c0 = half * 1024 + qt * 512
                            nc.tensor.matmul(
                                ps[:, qt * 512:(qt + 1) * 512],
                                lhsT=wt[:, i, 0:P],
                                rhs=xT[:, i, c0:c0 + 512],
                                start=(i == 0),
                                stop=(i == NEC - 1),
                            )
                    nc.scalar.copy(
                        dst[:, half * 1024:(half + 1) * 1024], ps[:]
                    )

            def rope0(src, kdst, qdst):
                # mb=0 RoPE on DVE (idle during PE-bound phase A);
                # kdst: full-width K output; qdst: zero-padded Q planes
                sh = rtmp.tile([P, S], bf16, tag="shuf",
                               name=f"sh0{'k' if qdst is None else 'q'}")
                nc.vector.stream_shuffle(sh[:], src[:], shuf_mask)
                nc.vector.tensor_mul(sh[:], sh[:], sin_t[:])
                if qdst is None:
                    nc.vector.tensor_mul(kdst[:], src[:], cos_t[:])
                    nc.vector.tensor_add(kdst[:], kdst[:], sh[:])
                else:
                    for par in range(2):
                        o0 = par * DH
                        nc.vector.tensor_mul(
                            qdst[o0:o0 + DH, par, :],
                            src[o0:o0 + DH, :],
                            cos_t[o0:o0 + DH, :],
                        )
                        nc.vector.tensor_add(
                            qdst[o0:o0 + DH, par, :],
                            qdst[o0:o0 + DH, par, :],
                            sh[o0:o0 + DH, :],
                        )

            proj0(wk_t, kcT[0], "k")
            rope0(kcT[0], kT[0], None)
            proj0(wq_t, qcT[0], "q")
            rope0(qcT[0], None, qz[0])
            proj0(wv_t, vT[0], "v")
            # V0 natural layout via PE transposes (fast + early; the
            # 1.2us-each serial xbar-transpose DMAs deliver too late
            # for h0's PV stream)
            for sb in range(NSB):
                ps = pr_ps.tile([P, P], bf16, tag="tp", name=f"tp0_{sb}")
                nc.tensor.transpose(
                    ps[:], vT[0][:, sb * P:(sb + 1) * P], ident[:]
                )
                for par in range(2):
                    nc.vector.tensor_copy(
                        vn[0][sb][par][:, 0:64],
                        ps[:, par * DH:(par + 1) * DH],
                    )

        # ---- Fused-B: solo-head attention windows + aux interleave ----
        with ExitStack() as bctx:
            sc_ps = bctx.enter_context(
                tc.tile_pool(name="sc_ps", bufs=2, space="PSUM")
            )
            ac_ps = bctx.enter_context(
                tc.tile_pool(name="ac_ps", bufs=1, space="PSUM")
            )
            aux_ps = bctx.enter_context(
                tc.tile_pool(name="aux_ps", bufs=2, space="PSUM")
            )
            ptp = bctx.enter_context(tc.tile_pool(name="ptp", bufs=4))
            dn = bctx.enter_context(tc.tile_pool(name="dn", bufs=2))
            rb = bctx.enter_context(tc.tile_pool(name="rb", bufs=2))

            def aux_steps():
                # mb=1 projections in [128,512] PSUM chunks (the 2
                # spare banks), RoPE(mb1) piecewise, V1 transposes.
                # Each yield is <= ~1.2us of engine time so attention
                # emission never stalls behind a big burst. Order: V1
                # first -- its 16 serial 1.2us xbar-transpose DMAs
                # (sync ring) have the earliest deadline (h2's PV
                # stream); K1/Q1 + RoPE follow (needed by h2's scores).
                for wt, dst, nm in ((wv_t, vT[1], "v1"),
                                    (wk_t, kcT[1], "k1"),
                                    (wq_t, qcT[1], "q1")):
                    for half in range(2):
                        for qt in range(2):
                            c0 = half * 1024 + qt * 512
                            ps = aux_ps.tile([P, 512], f32, tag="auxp",
                                             name=f"pj_{nm}_{c0}")
                            for i0 in (0, 4):
                                for i in range(i0, i0 + 4):
                                    nc.tensor.matmul(
                                        ps[:],
                                        lhsT=wt[:, i, P:2 * P],
                                        rhs=xT[:, i, c0:c0 + 512],
                                        start=(i == 0),
                                        stop=(i == NEC - 1),
                                    )
                                yield
                            nc.vector.tensor_copy(
                                dst[:, c0:c0 + 512], ps[:]
                            )
                            yield
                        if nm == "v1":
                            # V1 natural layout via DMA xbar transpose
                            # on the sync ring (idle during fused-B),
                            # emitted per proj half so the serial
                            # 1.2us-each stream starts ASAP
                            for sb in range(half * 8, half * 8 + 8):
                                for par in range(2):
                                    nc.sync.dma_start_transpose(
                                        vn[1][sb][par][:, 0:64],
                                        vT[1][par * DH:(par + 1) * DH,
                                              sb * P:(sb + 1) * P],
                                    )
                                    yield
                    if nm == "k1":
                        for c0 in range(0, S, 512):
                            sh = rb.tile([P, 512], bf16, tag="rsh",
                                         name=f"rshk{c0}")
                            nc.vector.stream_shuffle(
                                sh[:], kcT[1][:, c0:c0 + 512], shuf_mask
                            )
                            yield
                            nc.gpsimd.tensor_mul(
                                sh[:], sh[:], sin_t[:, c0:c0 + 512]
                            )
                            yield
                            nc.gpsimd.tensor_mul(
                                kT[1][:, c0:c0 + 512],
                                kcT[1][:, c0:c0 + 512],
                                cos_t[:, c0:c0 + 512],
                            )
                            yield
                            nc.gpsimd.tensor_add(
                                kT[1][:, c0:c0 + 512],
                                kT[1][:, c0:c0 + 512],
                                sh[:],
                            )
                            yield
                    elif nm == "q1":
                        for c0 in range(0, S, 512):
                            sh = rb.tile([P, 512], bf16, tag="rsh",
                                         name=f"rshq{c0}")
                            nc.vector.stream_shuffle(
                                sh[:], qcT[1][:, c0:c0 + 512], shuf_mask
                            )
                            yield
                            nc.vector.tensor_mul(
                                sh[:], sh[:], sin_t[:, c0:c0 + 512]
                            )
                            yield
                            for par in range(2):
                                o0 = par * DH
                                nc.vector.tensor_mul(
                                    qz[1][o0:o0 + DH, par, c0:c0 + 512],
                                    qcT[1][o0:o0 + DH, c0:c0 + 512],
                                    cos_t[o0:o0 + DH, c0:c0 + 512],
                                )
                                nc.vector.tensor_add(
                                    qz[1][o0:o0 + DH, par, c0:c0 + 512],
                                    qz[1][o0:o0 + DH, par, c0:c0 + 512],
                                    sh[o0:o0 + DH, :],
                                )
                                yield
            aux = aux_steps()
            aux_state = [True]

            def pump(n):
                for _ in range(n):
                    if not aux_state[0]:
                        return
                    try:
                        next(aux)
                    except StopIteration:
                        aux_state[0] = False

            def epilogue_copies(h, acc, c0, c1):
                # stage out^T + the denominator row out of PSUM (plain
                # DVE copies; these free the accumulator banks)
                q0 = (c0 // 1024) * 1024
                w = c1 - c0
                l0 = c0 - q0
                acb = dn.tile([DH, w], f32, tag="acb", name=f"acb{h}_{c0}")
                nc.vector.tensor_copy(acb[:], acc[0:DH, l0:l0 + w])
                den0 = dn.tile([1, w], f32, tag="den0", name=f"den0{h}_{c0}")
                nc.vector.tensor_copy(den0[:], acc[64:65, l0:l0 + w])
                return h, c0, c1, acb, den0

            def epilogue_norm(h, c0, c1, acb, den0):
                # single-pass approx reciprocal (~18 bits), partition
                # broadcast on GpSimd (reads partition 0 => rden is a
                # base-0 tile), one multiply
                mb, off = h // 2, (h % 2) * DH
                w = c1 - c0
                rden = dn.tile([1, w], f32, tag="rden", name=f"rden{h}_{c0}")
                nc.vector.reciprocal_approx_fast(rden[:], den0[:])
                rdb = dn.tile([DH, w], f32, tag="rdb", name=f"rdb{h}_{c0}")
                nc.gpsimd.partition_broadcast(rdb[:], rden[:])
                nc.vector.tensor_mul(
                    onrm_ap(off, off + DH, mb, c0, c1), acb[:], rdb[:]
                )

            deferred_norms = []
            for widx in range(8):
                h, pss = widx // 2, widx % 2
                mb, par = h // 2, h % 2
                q0 = pss * 1024
                acc = ac_ps.tile([65, 1024], f32, tag="acc",
                                 name=f"acc_{h}_{pss}")

                def issue_pv(ti, pt, lo, hi, acc=acc, h=h, mb=mb, q0=q0):
                    p0 = lo
                    while p0 < hi:
                        bk = p0 // 512
                        p1 = min(hi, (bk + 1) * 512)
                        nc.tensor.matmul(
                            acc[:, p0 - q0:p1 - q0],
                            lhsT=vn[mb][ti][h % 2][:],
                            rhs=pt[:, p0 - q0:p1 - q0],
                            start=(ti == 0),
                            stop=(ti == 4 * bk + 3),
                        )
                        p0 = p1

                pending = None
                nti = 8 if pss == 0 else NSB
                for ti in range(nti):
                    if ti == 2 and deferred_norms:
                        # previous window's normalize chains, emitted
                        # here so the PSUM-release semaphores are not
                        # queued behind the reciprocal work
                        for st in deferred_norms:
                            epilogue_norm(*st)
                        deferred_norms = []
                    if pss == 1 and ti == 13:
                        # acc bank for cols q0..q0+512 got its last PV
                        # at ti==11: normalize it mid-loop
                        epilogue_norm(
                            *epilogue_copies(h, acc, q0, q0 + 512)
                        )
                    pump(2 if widx < 4 else 1)
                    t0 = ti * P
                    lo = max(t0, q0)
                    hi = q0 + 1024
                    sc = sc_ps.tile([P, 1024], f32, tag="sc",
                                    name=f"sc_{h}_{pss}_{ti}")
                    p0 = lo
                    while p0 < hi:
                        p1 = min(hi, (p0 // 512 + 1) * 512)
                        nc.tensor.matmul(
                            sc[:, p0 - q0:p1 - q0],
                            lhsT=kT[mb][:, t0:t0 + P],
                            rhs=qz[mb][:, par, p0:p1],
                        )
                        p0 = p1
                    pt = ptp.tile([P, 1024], bf16, tag="pt",
                                  name=f"pt_{h}_{pss}_{ti}")
                    nc.scalar.activation(
                        pt[:, lo - q0:hi - q0],
                        sc[:, lo - q0:hi - q0],
                        AF.Exp,
                        scale=ATTN_SCALE,
                    )
                    if t0 >= q0:
                        # diagonal-block causal mask on DVE (on GpSimd
                        # it head-of-line blocks behind multi-us aux
                        # ops and stalls the PV pipeline)
                        nc.vector.tensor_mul(
                            pt[:, t0 - q0:t0 - q0 + P],
                            pt[:, t0 - q0:t0 - q0 + P],
                            msk_t[:],
                        )
                    if pending is not None:
                        issue_pv(*pending)
                    pending = (ti, pt, lo, hi)
                issue_pv(*pending)
                if pss == 1:
                    chunks = [(q0 + 512, q0 + 1024)]
                else:
                    chunks = [(q0, q0 + 1024)]
                staged = [epilogue_copies(h, acc, c0, c1)
                          for c0, c1 in chunks]
                if widx == 7:
                    for st in staged:
                        epilogue_norm(*st)
                else:
                    deferred_norms = staged
            pump(1000)  # drain any leftover aux work

        # ---- Phase C: output projection ----
        with ExitStack() as cctx:
            y_ps = cctx.enter_context(
                tc.tile_pool(name="y_ps", bufs=2, space="PSUM")
            )
            yo = cctx.enter_context(tc.tile_pool(name="yo", bufs=3))
            for sb_i in range(NSB):
                yp = y_ps.tile([P, E], f32, tag="yp")
                for mb in range(MB):
                    for half in range(2):
                        nc.tensor.matmul(
                            yp[:, half * 512:(half + 1) * 512],
                            lhsT=onrm_ap(0, P, mb, sb_i * P, (sb_i + 1) * P),
                            rhs=wo_t[:, mb, half * 512:(half + 1) * 512],
                            start=(mb == 0),
                            stop=(mb == MB - 1),
                        )
                ys = yo.tile([P, E], bf16, tag="ys")
                for half in range(2):
                    sl = slice(half * 512, (half + 1) * 512)
                    if (sb_i + half) % 2 == 0:
                        nc.vector.tensor_copy(ys[:, sl], yp[:, sl])
                    else:
                        nc.scalar.copy(ys[:, sl], yp[:, sl])
                    eng = nc.sync if half == 0 else nc.scalar
                    eng.dma_start(y[sb_i * P:(sb_i + 1) * P, sl], ys[:, sl])

    nc.compile()
    return nc


def get_program():
    global _PROG
    if _PROG is None:
        _PROG = _build_program()
    return _PROG


def make_in_maps(x, W_q, W_k, W_v, W_o):
    perm = _perm64()
    idx_local = (np.arange(DC) // 64) * 64 + perm[np.arange(DC) % 64]
    ang, sgn = _cos_sin_tiles()
    cos_np = np.cos(ang).astype(BF16)
    sin_np = (sgn * np.sin(ang)).astype(BF16)
    # scores tile is (t, q): keep t <= q -> upper triangular incl. diagonal
    cmask_np = np.triu(np.ones((P, P))).astype(BF16)
    in_maps = []
    for c in range(NCORES):
        b, hg = c // 4, c % 4
        base = hg * DC
        in_maps.append(
            dict(
                xbt=np.ascontiguousarray(x[b].T.astype(BF16)),
                wq=np.ascontiguousarray(W_q[:, base + idx_local].astype(BF16)),
                wk=np.ascontiguousarray(W_k[:, base + idx_local].astype(BF16)),
                wv=np.ascontiguousarray(W_v[:, base:base + DC].astype(BF16)),
                wo=np.ascontiguousarray(W_o[base:base + DC, :].astype(BF16)),
                cosr=cos_np,
                sinr=sin_np,
                cmask=cmask_np,
            )
        )
    return in_maps


def kernel(x, W_q, W_k, W_v, W_o, _trace=False, _trace_cores=None):
    from concourse.bass_utils import run_bass_kernel_spmd

    x = np.asarray(x, dtype=np.float32)
    W_q = np.asarray(W_q, dtype=np.float32)
    W_k = np.asarray(W_k, dtype=np.float32)
    W_v = np.asarray(W_v, dtype=np.float32)
    W_o = np.asarray(W_o, dtype=np.float32)

    nc = get_program()
    in_maps = make_in_maps(x, W_q, W_k, W_v, W_o)
    res = run_bass_kernel_spmd(
        nc,
        in_maps,
        list(range(NCORES)),
        trace=_trace,
        trace_cores=_trace_cores,
    )
    y = np.zeros((B, S, E), np.float32)
    for c in range(NCORES):
        y[c // 4] += res.results[c]["y"].astype(np.float32)
    if _trace:
        return y, res
    return y
